# revision 3
# baseline (speedup 1.0000x reference)
"""Trainium2 Bass kernel for quantized-MLP-with-LoRA (nn_MixedSparseTraditionalMLP).

v4: data-parallel over tokens (1024/core).

Host-side prep is pure layout (permutation / container narrowing / dtype cast,
no model arithmetic -- the (code-7.5)*scale dequant and all matmuls stay on
device):
- 4-bit weight codes (int32, values 0..15) are narrowed to int8 and
  pre-permuted into the contraction-major tile order the tensor engine needs,
  so the device does ZERO transposes.
- Per-block scales are pre-broadcast to one f16 scale per weight element in
  the same tile order (streamed from HBM, 2B/element).
- x1 is passed transposed as f16; y2 is returned transposed f16 and
  un-transposed + cast on host.

Device per up-slab: one 256KB int8 load + one 512KB f16 scale load + 2
full-rate DVE ops (cast/-7.5, *scale) + 35 matmuls. Down-slabs analogous.
x2 stays resident in SBUF (16MB); LoRA terms and biases are folded into the
PSUM accumulation groups; relu + up-bias applied on the PSUM->SBUF copy.
"""
import sys

if "/opt/trn_rl_repo" not in sys.path:
    sys.path.insert(0, "/opt/trn_rl_repo")

import numpy as np

import concourse.bass as bass
import concourse.mybir as mybir
import concourse.tile as tile
from concourse import bacc
from concourse.bass import ts, ds
from concourse.bass_utils import run_bass_kernel_spmd

F16 = mybir.dt.float16
F32 = mybir.dt.float32
I32 = mybir.dt.int32
I8 = mybir.dt.int8

NCORES = 8
T = 1024          # tokens per core
D = 2048
H = 8192
R = 16
P = 128
KD = D // P       # 16 k-subtiles for the up contraction
KH = H // P       # 64 k-subtiles / h-slabs
DM = D // P       # 16 d-slabs for the down projection
NT = T // 512     # 2 moving-operand tiles of 512 tokens
DQ = 4            # down dequant quarters per slab

TRACE = False
LAST_RESULTS = None


def _build():
    nc = bacc.Bacc("TRN2", target_bir_lowering=False, debug=False,
                   enable_asserts=False, num_devices=NCORES)

    x1tc = nc.dram_tensor("x1tc", [D, T], F16, kind="ExternalInput").ap()
    wupT8 = nc.dram_tensor("wupT8", [P, KH, KD * P], I8,
                           kind="ExternalInput").ap()
    supX = nc.dram_tensor("supX", [P, KH, KD * P], F16,
                          kind="ExternalInput").ap()
    bupT_h = nc.dram_tensor("bupT_h", [P, KH], F16, kind="ExternalInput").ap()
    a1f_h = nc.dram_tensor("a1f_h", [P, KD, R], F16, kind="ExternalInput").ap()
    b1c = nc.dram_tensor("b1c", [R, H], F16, kind="ExternalInput").ap()
    wdnT8 = nc.dram_tensor("wdnT8", [P, DM, KH * P], I8,
                           kind="ExternalInput").ap()
    sdnX = nc.dram_tensor("sdnX", [P, DM, KH * P], F16,
                          kind="ExternalInput").ap()
    a2f_h = nc.dram_tensor("a2f_h", [P, KH, R], F16, kind="ExternalInput").ap()
    b2aug_h = nc.dram_tensor("b2aug_h", [R + 1, D], F16,
                             kind="ExternalInput").ap()
    # output: y2 transposed, f16; host un-transposes and casts
    y2t = nc.dram_tensor("y2t", [D, T], F16, kind="ExternalOutput").ap()

    with tile.TileContext(nc) as tc:
        with tc.tile_pool(name="const", bufs=1) as cp, \
             tc.tile_pool(name="psum", bufs=4, space="PSUM") as pp, \
             tc.tile_pool(name="psum_vt", bufs=1, space="PSUM") as pvt:

            # ---- long-lived tiles ----
            x2t = cp.tile([P, KH, T], F16, tag="x2t")       # resident hidden act
            a2f = cp.tile([P, KH, R], F16, tag="a2f")
            v1t = cp.tile([R + 1, T], F16, tag="v1t")
            # row R reads 1.0 (folds b_down into the lora matmul); rows 0..R-1
            # are overwritten with v2^T after the up phase
            nc.any.memset(v1t[:], 1.0)
            nc.sync.dma_start(a2f[:], a2f_h)

            vt_ps = [pvt.tile([R, 512], F32, tag=f"vt{i}", name=f"vt{i}")
                     for i in range(NT)]

            with tc.tile_pool(name="upc", bufs=1) as up, \
                 tc.tile_pool(name="qstage", bufs=3) as qp, \
                 tc.tile_pool(name="sstage", bufs=3) as sp_, \
                 tc.tile_pool(name="wup", bufs=3) as wp:

                x1t = up.tile([P, KD, T], F16, tag="x1t")
                a1f = up.tile([P, KD, R], F16, tag="a1f")
                bupT = up.tile([P, KH], F16, tag="bupT")
                utf = up.tile([R, T], F16, tag="utf")

                nc.sync.dma_start(x1t[:], x1tc.rearrange("(j p) t -> p j t", p=P))
                nc.sync.dma_start(a1f[:], a1f_h)
                nc.sync.dma_start(bupT[:], bupT_h)

                # ---- uT = (x1 @ A1)^T : [R, T] ----
                for tt in range(NT):
                    ups = pp.tile([R, 512], F32, tag="mm")
                    for j in range(KD):
                        nc.tensor.matmul(ups[:], a1f[:, j, :], x1t[:, j, ts(tt, 512)],
                                         start=(j == 0), stop=(j == KD - 1))
                    nc.scalar.copy(utf[:, ts(tt, 512)], ups[:])

                # ---- UP: one 128-row slab of H per step ----
                for k in range(KH):
                    qst = qp.tile([P, KD * P], I8, tag="qst")
                    nc.sync.dma_start(qst[:], wupT8[:, k, :])
                    sx = sp_.tile([P, KD * P], F16, tag="sx")
                    nc.sync.dma_start(sx[:], supX[:, k, :])
                    wt = wp.tile([P, KD, P], F16, tag="wupt")
                    wtf = wt[:].rearrange("p j h -> p (j h)")
                    nc.vector.tensor_scalar_add(wtf, qst[:], -7.5)
                    nc.vector.tensor_tensor(wtf, wtf, sx[:],
                                            mybir.AluOpType.mult)

                    # b1 slab [R, 128] loaded on the fly (f16, host-cast)
                    b1s = qp.tile([R, P], F16, tag="b1s")
                    nc.gpsimd.dma_start(b1s[:], b1c[:, ts(k, P)])

                    for tt in range(NT):
                        ps = pp.tile([P, 512], F32, tag="mm")
                        for j in range(KD):
                            nc.tensor.matmul(ps[:], wt[:, j, :],
                                             x1t[:, j, ts(tt, 512)],
                                             start=(j == 0), stop=False)
                        nc.tensor.matmul(ps[:], b1s[:], utf[:, ts(tt, 512)],
                                         start=False, stop=True)
                        nc.scalar.activation(x2t[:, k, ts(tt, 512)], ps[:],
                                             mybir.ActivationFunctionType.Relu,
                                             bias=bupT[:, k:k + 1], scale=1.0)
                        nc.tensor.matmul(vt_ps[tt][:], a2f[:, k, :],
                                         x2t[:, k, ts(tt, 512)],
                                         start=(k == 0), stop=(k == KH - 1),
                                         skip_group_check=True)

            for tt in range(NT):
                nc.scalar.copy(v1t[:R, ts(tt, 512)], vt_ps[tt][:])

            # ---- DOWN: dequantized w_down^T slabs stationary, x2 streams ----
            with tc.tile_pool(name="dnc", bufs=1) as dc, \
                 tc.tile_pool(name="wdn", bufs=2) as wd, \
                 tc.tile_pool(name="dqstage", bufs=5) as dqp, \
                 tc.tile_pool(name="dsstage", bufs=5) as dsp, \
                 tc.tile_pool(name="yout", bufs=2) as yp:

                b2p = dc.tile([R + 1, D], F16, tag="b2p")
                nc.sync.dma_start(b2p[:], b2aug_h)

                CW = KH // DQ * P    # 2048 codes per dequant quarter

                for m in range(DM):
                    wdt = wd.tile([P, KH, P], F16, tag="wdt")  # [h_part, hsub, d]
                    for q in range(DQ):
                        qst = dqp.tile([P, CW], I8, tag="qst")
                        nc.sync.dma_start(qst[:], wdnT8[:, m, ts(q, CW)])
                        sx = dsp.tile([P, CW], F16, tag="sx")
                        nc.sync.dma_start(sx[:], sdnX[:, m, ts(q, CW)])
                        wq = wdt[:, ds(16 * q, 16), :].rearrange("p a b -> p (a b)")
                        nc.vector.tensor_scalar_add(wq, qst[:], -7.5)
                        nc.vector.tensor_tensor(wq, wq, sx[:],
                                                mybir.AluOpType.mult)

                    yo = yp.tile([P, T], F16, tag="yo")
                    for tt in range(NT):
                        psd = pp.tile([P, 512], F32, tag="mm")
                        for hsub in range(KH):
                            nc.tensor.matmul(psd[:], wdt[:, hsub, :],
                                             x2t[:, hsub, ts(tt, 512)],
                                             start=(hsub == 0), stop=False)
                        nc.tensor.matmul(psd[:], b2p[:, ts(m, P)],
                                         v1t[:, ts(tt, 512)],
                                         start=False, stop=True)
                        nc.scalar.copy(yo[:, ts(tt, 512)], psd[:])
                    nc.gpsimd.dma_start(y2t[ts(m, P), :], yo[:])

    nc.compile()
    return nc


_NC = None
_PREP_CACHE = {}


def _prep_shared(w_up_q, w_up_scale, b_up, w_up_lora_a, w_up_lora_b,
                 w_down_q, w_down_scale, b_down, w_down_lora_a, w_down_lora_b):
    """Host-side layout prep: permutation / narrowing / casts only."""
    f16 = np.float16
    wup = np.asarray(w_up_q)
    key = (id(wup), wup.shape)
    hit = _PREP_CACHE.get(key)
    if hit is not None:
        return hit

    sup = np.asarray(w_up_scale, np.float32)
    sdn = np.asarray(w_down_scale, np.float32)
    wdn = np.asarray(w_down_q)

    # codes: [p, k, j*128+h'] = w_up_q[k*128+h', j*128+p], int8 container
    wupT8 = np.ascontiguousarray(
        wup.reshape(KH, P, KD, P).transpose(3, 0, 2, 1)
        .reshape(P, KH, KD * P).astype(np.int8))
    wdnT8 = np.ascontiguousarray(
        wdn.reshape(DM, P, KH, P).transpose(3, 0, 2, 1)
        .reshape(P, DM, KH * P).astype(np.int8))

    # scales, one f16 per weight element in the same tile order:
    # supX[p, k, j*128+h'] = sup[k*128+h', (j*128+p)//64]
    supT = sup.astype(f16).reshape(KH, P, D // 64)        # [k, h', b]
    bidx = (np.arange(KD)[:, None] * P + np.arange(P)[None, :]) // 64  # [j, p]
    # -> [p, k, j, h']
    supX = np.ascontiguousarray(
        supT[:, :, bidx].transpose(3, 0, 2, 1).reshape(P, KH, KD * P))
    sdnT = sdn.astype(f16).reshape(DM, P, H // 64)        # [m, d', b]
    bidx2 = (np.arange(KH)[:, None] * P + np.arange(P)[None, :]) // 64  # [hs, p]
    sdnX = np.ascontiguousarray(
        sdnT[:, :, bidx2].transpose(3, 0, 2, 1).reshape(P, DM, KH * P))

    shared = {
        "wupT8": wupT8, "wdnT8": wdnT8, "supX": supX, "sdnX": sdnX,
        "bupT_h": np.ascontiguousarray(
            np.asarray(b_up, np.float32).reshape(KH, P).T.astype(f16)),
        "a1f_h": np.ascontiguousarray(
            np.asarray(w_up_lora_a, np.float32).reshape(KD, P, R)
            .transpose(1, 0, 2).astype(f16)),
        "a2f_h": np.ascontiguousarray(
            np.asarray(w_down_lora_a, np.float32).reshape(KH, P, R)
            .transpose(1, 0, 2).astype(f16)),
        "b1c": np.ascontiguousarray(
            np.asarray(w_up_lora_b, np.float32).astype(f16)),
        "b2aug_h": np.ascontiguousarray(np.vstack(
            [np.asarray(w_down_lora_b, np.float32),
             np.asarray(b_down, np.float32)[None, :]]).astype(f16)),
    }
    _PREP_CACHE.clear()
    _PREP_CACHE[key] = shared
    return shared


def make_in_maps(x1, w_up_q, w_up_scale, b_up, w_up_lora_a, w_up_lora_b,
                 w_down_q, w_down_scale, b_down, w_down_lora_a, w_down_lora_b):
    x1 = np.ascontiguousarray(np.asarray(x1, dtype=np.float32))
    xf = x1.reshape(-1, D)
    shared = _prep_shared(w_up_q, w_up_scale, b_up, w_up_lora_a, w_up_lora_b,
                          w_down_q, w_down_scale, b_down, w_down_lora_a,
                          w_down_lora_b)
    return [
        {"x1tc": np.ascontiguousarray(xf[c * T:(c + 1) * T].T.astype(np.float16)),
         **shared}
        for c in range(NCORES)]


def kernel(x1, w_up_q, w_up_scale, b_up, w_up_lora_a, w_up_lora_b,
           w_down_q, w_down_scale, b_down, w_down_lora_a, w_down_lora_b):
    global _NC, LAST_RESULTS
    if _NC is None:
        _NC = _build()

    x1 = np.ascontiguousarray(np.asarray(x1, dtype=np.float32))
    B, S, _ = x1.shape
    in_maps = make_in_maps(x1, w_up_q, w_up_scale, b_up, w_up_lora_a,
                           w_up_lora_b, w_down_q, w_down_scale, b_down,
                           w_down_lora_a, w_down_lora_b)

    res = run_bass_kernel_spmd(_NC, in_maps, core_ids=list(range(NCORES)),
                               trace=TRACE)
    LAST_RESULTS = res
    out = np.concatenate(
        [np.ascontiguousarray(res.results[c]["y2t"].T).astype(np.float32)
         for c in range(NCORES)], axis=0)
    return out.reshape(B, S, D)
